# revision 1
# baseline (speedup 1.0000x reference)
"""Trainium2 Bass kernel for NT-Xent style contrastive loss.

Math (B=4096, D=128, T=0.25), with z = row-normalized emb:
  S = z_i @ z_j^T   [B, B]
  loss = (1/2B) * sum_r [ -2*S[r,r]/T + ln(sum_c exp(S[r,c]/T))
                                      + ln(sum_c exp(S[c,r]/T)) ]
exp(S) is computed exactly once; its row sums and column sums feed the two
ln branches (branch-b row sums of exp(S^T) == column sums of exp(S)).

Sharding: 2D. Core (rb, ch), rb = core//2, ch = core%2, owns the
[1024 rows x 2048 cols] block. All inputs are plain row slices (no host
rotation, no collectives; the cross-core reduction is 8 tiny partial
tensors summed on the host).

Orientation: S^T chunks [128 cols, rows]: stationary = scaled column tile
zcjT [d, 128 c], moving = normalized zT_i [d, 1024 r]. Both norm factors
are pre-applied to the matmul operands (rows: z_i = ai/n_i; cols:
zcj = cj * 4/n_c, absorbing 1/T), so PSUM holds s/T directly and the exp
has constant scale — ACT exp instructions span 2 c-tiles [128, 2048]
(8 instructions, ~2.0us each: the ACT stream is the main-loop floor).

Per-core dataflow:
  - loads: ai + oj fp32 on HWDGE lanes (parallel), cj bf16-cast via the
    SWDGE FIFO in 4+4+8-tile chunks (FIFO consumers wait for everything
    queued before them, so nothing else may ride that queue)
  - DVE: row/col norm stats (square + reduce), ln/exp on ACT, scales via
    stride-0 broadcast tensor_tensor
  - xbar transposes: dummy (absorbs the copy->transpose mode switch),
    zT_i, zcjT in 2 groups
  - main loop, 8 chunks of 2 c-tiles: 4 matmuls N=512 -> ps [128, 2048]
    (2 PSUM tiles double-buffered = all 8 banks), ACT Exp -> eb bf16,
    then per c-tile one DVE scalar_tensor_tensor: Esum += eb-tile with
    accum_out = running column sums (host takes telescoping differences)
  - tail: 2 mask-matmuls reduce Esum partitions -> psR [2, 512] row-sum
    partials (PSUM slot reused from the main pool), diag partials from
    GPSIMD dot products, 2 HWDGE output DMAs
Host: sum row-sum partials over core pairs and column sums over quads,
ln both, add diag partials, divide by 2B.
"""

import numpy as np

B = 4096
D = 128
P = 128
NCORES = 8
RB = 1024                  # rows per core
CB = 2048                  # cols per core
RT = RB // P               # 8 row t-tiles
CT = CB // P               # 16 col t-tiles
TEMP = 0.25
LN4 = float(np.log(4.0))

_cache = {}


def _build_bass():
    import concourse.bass as bass
    import concourse.mybir as mybir
    import concourse.tile as tile
    from concourse.bass import broadcast_tensor_aps
    from concourse.tile_rust import add_dep_helper

    f32 = mybir.dt.float32
    bf16 = mybir.dt.bfloat16
    AF = mybir.ActivationFunctionType
    ALU = mybir.AluOpType
    AX = mybir.AxisListType

    nc = bass.Bass("TRN2")
    ai_d = nc.dram_tensor("emb_i_blk", [RB, D], f32, kind="ExternalInput")
    cj_d = nc.dram_tensor("emb_j_cols", [CB, D], f32, kind="ExternalInput")
    oj_d = nc.dram_tensor("emb_j_own", [RB, D], f32, kind="ExternalInput")
    out_cs = nc.dram_tensor("colsum", [P, CT + 1], f32, kind="ExternalOutput")
    out_rs = nc.dram_tensor("rowsum", [2, 512], f32, kind="ExternalOutput")

    ai_t = ai_d.rearrange("(t p) d -> p t d", p=P)   # [128, 8, 128]
    cj_t = cj_d.rearrange("(t p) d -> p t d", p=P)   # [128, 16, 128]
    oj_t = oj_d.rearrange("(t p) d -> p t d", p=P)   # [128, 8, 128]

    with tile.TileContext(nc) as tc:
        with (
            tc.tile_pool(name="persist", bufs=1) as persist,
            tc.tile_pool(name="scratch", bufs=4) as scratch,
            tc.tile_pool(name="ebuf", bufs=2) as ebuf,
            tc.tile_pool(name="psmain", bufs=2, space="PSUM") as psmain,
        ):
            cj = persist.tile([P, CT, D], bf16, tag="cj")
            zcj = persist.tile([P, CT, D], bf16, tag="zcj")
            zcjT = persist.tile([P, CT, D], bf16, tag="zcjT")
            ai = persist.tile([P, RT, D], f32, tag="ai")
            z_i = persist.tile([P, RT, D], bf16, tag="z_i")
            zT_i = persist.tile([P, RT, D], bf16, tag="zT_i")
            oj = persist.tile([P, RT, D], f32, tag="oj")
            mask = persist.tile([P, 16], bf16, tag="mask")
            zb = persist.tile([P, 1], f32, tag="zb")
            b_ln4 = persist.tile([P, 1], f32, tag="b_ln4")
            dummy_out = persist.tile([P, 16], bf16, tag="dummy_out")

            n2i = persist.tile([P, RT], f32, tag="n2i")
            invi = persist.tile([P, RT, 1], bf16, tag="invi")
            n2o = persist.tile([P, RT], f32, tag="n2o")
            inv4o = persist.tile([P, RT], f32, tag="inv4o")
            n2c = persist.tile([P, CT], f32, tag="n2c")
            inv4c = persist.tile([P, CT, 1], bf16, tag="inv4c")
            cs_sb = persist.tile([P, CT + 1], f32, tag="cs_sb")
            rs_sb = persist.tile([2, 512], f32, tag="rs_sb")
            ezero = persist.tile([P, RB], bf16, tag="ezero")
            esum = [
                persist.tile([P, RB], bf16, name="esum0", tag="esum0"),
                persist.tile([P, RB], bf16, name="esum1", tag="esum1"),
            ]

            nc.vector.memset(mask, 0.0)
            nc.vector.memset(mask[:, 8:9], 1.0)
            nc.vector.memset(zb, 0.0)
            nc.vector.memset(b_ln4, LN4)
            nc.vector.memset(ezero, 0.0)
            tblw = scratch.tile([P, 1], f32, tag="tblw")
            nc.scalar.activation(tblw, zb, AF.Ln, bias=b_ln4)

            # ---- loads: ai fp32 via HWDGE (parallel queue); cj in
            # 4+4+8-tile chunks + oj via the SWDGE cast queue ----
            nc.sync.dma_start(out=ai, in_=ai_t)
            nc.sync.dma_start(out=oj, in_=oj_t)
            nc.gpsimd.dma_start(out=cj[:, 0:4, :], in_=cj_t[:, 0:4, :])
            nc.gpsimd.dma_start(out=cj[:, 4:8, :], in_=cj_t[:, 4:8, :])
            nc.gpsimd.dma_start(out=cj[:, 8:16, :], in_=cj_t[:, 8:16, :])

            # ---- norm stats: i then j groups (sq+reduce interleaved so
            # DVE fills the ACT ln/exp bubbles), broadcast-AP scales ----
            sqi = scratch.tile([P, RT, D], bf16, tag="sqi")
            nc.vector.tensor_mul(sqi, ai, ai)
            nc.vector.tensor_reduce(out=n2i, in_=sqi, axis=AX.X, op=ALU.add)
            lgi = scratch.tile([P, RT], f32, tag="lgi")
            nc.scalar.activation(lgi, n2i, AF.Ln, bias=zb)
            nc.scalar.activation(invi[:, :, 0], lgi, AF.Exp, scale=-0.5, bias=zb)

            JG = ((0, 4), (4, 16))

            def jstats(g):
                lo, hi = JG[g]
                ts = slice(lo, hi)
                sq = scratch.tile([P, hi - lo, D], bf16, name=f"sqj{g}", tag=f"sqj{g}")
                nc.vector.tensor_mul(sq, cj[:, ts, :], cj[:, ts, :])
                nc.vector.tensor_reduce(out=n2c[:, ts], in_=sq, axis=AX.X, op=ALU.add)
                lgc = scratch.tile([P, hi - lo], f32, name=f"lgc{g}", tag=f"lgc{g}")
                nc.scalar.activation(lgc, n2c[:, ts], AF.Ln, bias=zb)
                nc.scalar.activation(
                    inv4c[:, ts, 0], lgc, AF.Exp, scale=-0.5, bias=b_ln4
                )

            def jscale(g):
                lo, hi = JG[g]
                ts = slice(lo, hi)
                a_ap, b_ap = broadcast_tensor_aps(cj[:, ts, :], inv4c[:, ts, :])
                nc.vector.tensor_tensor(
                    out=zcj[:, ts, :], in0=a_ap, in1=b_ap, op=ALU.mult
                )

            a_ap, b_ap = broadcast_tensor_aps(ai[:, :, :], invi[:, :, :])
            nc.vector.tensor_tensor(out=z_i[:, :, :], in0=a_ap, in1=b_ap, op=ALU.mult)
            for g in range(2):
                jstats(g)
                jscale(g)

            # ---- transposes ----
            dummy_inst = nc.sync.dma_start_transpose(
                out=dummy_out, in_=cj[0:16, 0, :]
            )
            tzi = nc.sync.dma_start_transpose(out=zT_i, in_=z_i)
            add_dep_helper(tzi.ins, dummy_inst.ins, False, "xpose after dummy")
            for lo, hi in ((0, 4), (4, 16)):
                tj = nc.sync.dma_start_transpose(
                    out=zcjT[:, lo:hi, :], in_=zcj[:, lo:hi, :]
                )
                add_dep_helper(tj.ins, dummy_inst.ins, False, "xpose after dummy")

            # ---- diag stats (GPSIMD — off the DVE ramp critical path) ----
            sqo = scratch.tile([P, RT, D], bf16, tag="sqo")
            nc.gpsimd.tensor_mul(sqo, oj, oj)
            nc.vector.tensor_reduce(out=n2o, in_=sqo, axis=AX.X, op=ALU.add)
            lgo = scratch.tile([P, RT], f32, tag="lgo")
            nc.scalar.activation(lgo, n2o, AF.Ln, bias=zb)
            nc.scalar.activation(inv4o, lgo, AF.Exp, scale=-0.5, bias=b_ln4)
            ddt = scratch.tile([P, RT, D], bf16, tag="ddt")
            nc.gpsimd.tensor_mul(ddt, z_i, oj)
            dvec = persist.tile([P, RT], f32, tag="dvec")
            nc.vector.tensor_reduce(out=dvec, in_=ddt, axis=AX.X, op=ALU.add)

            zTi_flat = zT_i.rearrange("p t d -> p (t d)")

            # ---- main loop: 8 chunks of 2 c-tiles ----
            for k in range(8):
                ps = psmain.tile([P, 2048], f32, tag="ps")
                for sub in range(2):
                    tcc = 2 * k + sub
                    for q in range(2):
                        nc.tensor.matmul(
                            ps[:, sub * 1024 + q * 512 : sub * 1024 + (q + 1) * 512],
                            zcjT[:, tcc, :],
                            zTi_flat[:, q * 512 : (q + 1) * 512],
                            start=True,
                            stop=True,
                        )
                eb = ebuf.tile([P, 2048], bf16, tag="eb")
                if k == 7:
                    eb_last = eb
                nc.scalar.activation(eb, ps, AF.Exp, bias=zb)
                for sub in range(2):
                    tcc = 2 * k + sub
                    prev = ezero if tcc == 0 else esum[(tcc - 1) % 2]
                    nc.vector.scalar_tensor_tensor(
                        out=esum[tcc % 2],
                        in0=eb[:, sub * RB : (sub + 1) * RB],
                        scalar=1.0,
                        in1=prev,
                        op0=ALU.mult,
                        op1=ALU.add,
                        accum_out=cs_sb[:, tcc : tcc + 1],
                    )

            # ---- tail: rowsum partials. Tile 15's rows come straight
            # from its exp output (ready after chunk 7's ACT, before the
            # final STT); tiles 0-14 from the Esum snapshot after tile 14.
            e14 = esum[(CT - 2) % 2]
            psR_full = psmain.tile([P, 2048], f32, tag="ps")
            psR = psR_full[0:2, 0:512]
            for q in range(2):
                nc.tensor.matmul(
                    psR,
                    mask[:, 8 - q : 10 - q],
                    eb_last[:, RB + q * 512 : RB + (q + 1) * 512],
                    start=(q == 0),
                    stop=False,
                )
            for q in range(2):
                nc.tensor.matmul(
                    psR,
                    mask[:, 8 - q : 10 - q],
                    e14[:, q * 512 : (q + 1) * 512],
                    start=False,
                    stop=(q == 1),
                )

            dsc = scratch.tile([P, RT], f32, tag="dsc")
            nc.vector.tensor_mul(dsc, dvec, inv4o)
            nc.vector.tensor_scalar_mul(dsc, dsc, -2.0)
            nc.vector.tensor_reduce(
                out=cs_sb[:, CT : CT + 1], in_=dsc, axis=AX.X, op=ALU.add
            )
            nc.vector.tensor_copy(rs_sb, psR)

            nc.sync.dma_start(out=out_cs[:, :], in_=cs_sb)
            nc.sync.dma_start(out=out_rs[:, :], in_=rs_sb)

    return nc


def _split_multi_waits(bir: bytes) -> bytes:
    """The walrus build in this container accepts only ONE sync-wait per
    compute/DMA instruction. Tile emits up to three. Move all but one wait
    onto standalone EventSemaphore instructions inserted just before the
    offender on the same engine queue."""
    import json

    d = json.loads(bir)
    n_split = 0
    for fn in d["functions"]:
        for blk in fn["blocks"]:
            new_insts = []
            for ins in blk["instructions"]:
                si = ins.get("sync_info")
                waits = (si or {}).get("on_wait") or []
                if len(waits) > 1:
                    for w in waits[:-1]:
                        ev = {
                            "debug": ins.get("debug", 0),
                            "engine": ins["engine"],
                            "ins": [],
                            "outs": [],
                            "name": f"{ins['name']}_wsplit{n_split}",
                            "opcode": "EventSemaphore",
                            "sync_info": {"on_update": [], "on_wait": [w]},
                        }
                        n_split += 1
                        new_insts.append(ev)
                    si["on_wait"] = [waits[-1]]
                new_insts.append(ins)
            blk["instructions"] = new_insts
    return json.dumps(d).encode()


def kernel(emb_i: np.ndarray, emb_j: np.ndarray) -> np.ndarray:
    from concourse.bass_utils import run_bass_kernel_spmd

    if "nc" not in _cache:
        nc = _build_bass()
        fixed = _split_multi_waits(nc.to_json_bytes())
        nc.to_json_bytes = lambda: fixed
        _cache["nc"] = nc
    nc = _cache["nc"]

    emb_i = np.ascontiguousarray(emb_i, dtype=np.float32)
    emb_j = np.ascontiguousarray(emb_j, dtype=np.float32)
    in_maps = []
    for c in range(NCORES):
        rb, ch = c // 2, c % 2
        in_maps.append(
            {
                "emb_i_blk": emb_i[rb * RB : (rb + 1) * RB],
                "emb_j_cols": emb_j[ch * CB : (ch + 1) * CB],
                "emb_j_own": emb_j[rb * RB : (rb + 1) * RB],
            }
        )

    import os

    trace = bool(os.environ.get("KERNEL_TRACE"))
    res = run_bass_kernel_spmd(
        nc, in_maps, core_ids=list(range(NCORES)), trace=trace
    )
    _cache["last_res"] = res

    # host combine
    dtot = np.float64(0.0)
    cs_total = np.zeros(B, dtype=np.float64)
    rs_total = np.zeros(B, dtype=np.float64)
    for c, r in enumerate(res.results):
        rb, ch = c // 2, c % 2
        cs = r["colsum"].astype(np.float64)
        # cs[:, tc] are RUNNING column sums; telescoping differences
        run = cs[:, :CT]
        per_tile = np.diff(
            np.concatenate([np.zeros((P, 1)), run], axis=1), axis=1
        )
        # per_tile[p, tc] covers global col  ch*CB + tc*128 + p
        cs_total[ch * CB : (ch + 1) * CB] += per_tile.T.reshape(CB)
        dtot += np.float64(cs[:, CT].sum())
        rs_total[rb * RB : (rb + 1) * RB] += (
            r["rowsum"].reshape(RB).astype(np.float64)
        )
    total = dtot + np.log(rs_total).sum() + np.log(cs_total).sum()
    loss = total / (2 * B)
    return np.array(loss, dtype=np.float32)



# revision 5
# speedup vs baseline: 1.1846x; 1.1846x over previous
"""Trainium2 Bass kernel for NT-Xent style contrastive loss (v2).

Math (B=4096, D=128, T=0.25), z = row-normalized emb:
  S = z_i @ z_j^T   [B, B]
  loss = (1/2B) * sum_r [ -2*S[r,r]/T + ln(sum_c exp(S[r,c]/T))
                                      + ln(sum_c exp(S[c,r]/T)) ]

Sharding: core (rb, ch), rb = core//2, ch = core%2: rows rb*1024 of emb_i,
cols ch*2048 of emb_j. All loads are HWDGE fp32 with "(p t) d" tiling so
each DMA descriptor is 2-4KB contiguous (row g of a block sits at
partition g//T, tile g%T; host unpermutes the tiny outputs).

Orientation: ps chunk k = [128 r (row-tile k), 2048 c free], 8 chunks.
  stationary = aibT tile k  (RAW bf16-cast emb_i, transposed — row stats
               are NOT on the matmul critical path)
  moving     = zcjT [d, 2048] (cols pre-scaled by 4/|c| — absorbs 1/T)
The EXP applies the row norm via its per-partition scale AP
(scale = invi[:,k], 1/|a_r|) and its accumulator emits the ROW SUMS
directly (accum_out -> rs[:, k]).  Column-sum partials come from a
bf16 TT esum chain (DVE 2x mode, one op per chunk) reduced over
partitions by 4 tail mask-matmuls -> [2, 2048] PSUM.

Engine split:
  ACT: ai cast, cj squares, ln/exp stat scalars, 8 big EXP+accum, cs copy
  DVE: sq-i (2x), 3 free-axis reduces + 2 col scales (preamble),
       esum copy+TT chain (2x/4x), diag reduces squeezed mid-loop
  GP : diag elementwise mults (sq-o, ddt) — GP cannot reduce free axis
  PE : 32 main matmuls N=512 + 4 tail mask-matmuls
  XBAR: dummy + aibT + zcjT in 2 halves

Host: unpermute + sum partials, ln, diag terms, /2B.
"""

import numpy as np

B = 4096
D = 128
P = 128
NCORES = 8
RB = 1024                  # rows per core
CB = 2048                  # cols per core
RT = RB // P               # 8 row tiles
CT = CB // P               # 16 col tiles
TEMP = 0.25
LN4 = float(np.log(4.0))

_cache = {}


def _build_bass():
    import concourse.bass as bass
    import concourse.mybir as mybir
    import concourse.tile as tile
    from concourse.bass import broadcast_tensor_aps
    from concourse.tile_rust import add_dep_helper

    f32 = mybir.dt.float32
    bf16 = mybir.dt.bfloat16
    AF = mybir.ActivationFunctionType
    ALU = mybir.AluOpType
    AX = mybir.AxisListType

    nc = bass.Bass("TRN2")
    ai_d = nc.dram_tensor("emb_i_blk", [RB, D], f32, kind="ExternalInput")
    cj_d = nc.dram_tensor("emb_j_cols", [CB, D], f32, kind="ExternalInput")
    oj_d = nc.dram_tensor("emb_j_own", [RB, D], f32, kind="ExternalInput")
    out_rs = nc.dram_tensor("rowsum", [P, RT], f32, kind="ExternalOutput")
    out_cs = nc.dram_tensor("colsum", [2, CB], f32, kind="ExternalOutput")
    out_dg = nc.dram_tensor("diag", [P, 3 * RT], f32, kind="ExternalOutput")

    # (p t) d: partition p holds rows p*T .. p*T+T-1 -> 2-4KB descriptors
    ai_t = ai_d.rearrange("(p t) d -> p t d", p=P)   # row g = p*8 + t
    cj_t = cj_d.rearrange("(p t) d -> p t d", p=P)   # col g = p*16 + t
    oj_t = oj_d.rearrange("(p t) d -> p t d", p=P)

    with tile.TileContext(nc) as tc:
        with (
            tc.tile_pool(name="persist", bufs=1) as persist,
            tc.tile_pool(name="scratch", bufs=4) as scratch,
            tc.tile_pool(name="ebuf", bufs=2) as ebuf,
            tc.tile_pool(name="psmain", bufs=2, space="PSUM") as psmain,
        ):
            ai = persist.tile([P, RT, D], f32, tag="ai")
            cj = persist.tile([P, CT, D], f32, tag="cj")
            oj = persist.tile([P, RT, D], f32, tag="oj")
            aib = persist.tile([P, RT, D], bf16, tag="aib")
            aibT = persist.tile([P, RT, D], bf16, tag="aibT")
            sqj = persist.tile([P, CT, D], bf16, tag="sqj")
            sqi = persist.tile([P, RT, D], bf16, tag="sqi")
            zcj = persist.tile([P, CT, D], bf16, tag="zcj")
            zcjT = persist.tile([P, CT, D], bf16, tag="zcjT")
            n2c = persist.tile([P, CT], f32, tag="n2c")
            n2i = persist.tile([P, RT], f32, tag="n2i")
            inv4c = persist.tile([P, CT, 1], f32, tag="inv4c")
            dg = persist.tile([P, 3 * RT], f32, tag="dg")  # rvec|n2o|invi
            rs_sb = persist.tile([P, RT], f32, tag="rs_sb")
            cs_sb = persist.tile([2, CB], f32, tag="cs_sb")
            esum = [
                persist.tile([P, CB], bf16, name="esum0", tag="esum0"),
                persist.tile([P, CB], bf16, name="esum1", tag="esum1"),
            ]
            zb = persist.tile([P, 1], f32, tag="zb")
            b_ln4 = persist.tile([P, 1], f32, tag="b_ln4")
            mask2 = persist.tile([P, 2], bf16, tag="mask2")
            dxin = persist.tile([16, D], bf16, tag="dxin")
            dxout = persist.tile([P, 16], bf16, tag="dxout")

            rvec = dg[:, 0:RT]
            n2o = dg[:, RT:2 * RT]
            invi = dg[:, 2 * RT:3 * RT]

            # ---- loads (HWDGE fp32, big descriptors) ----
            nc.sync.dma_start(out=ai, in_=ai_t)
            nc.sync.dma_start(out=cj[:, 0:8, :], in_=cj_t[:, 0:8, :])
            nc.sync.dma_start(out=cj[:, 8:16, :], in_=cj_t[:, 8:16, :])
            nc.sync.dma_start(out=oj, in_=oj_t)

            # ---- tiny constants ----
            nc.vector.memset(zb, 0.0)
            nc.vector.memset(b_ln4, LN4)
            nc.vector.memset(mask2, 1.0)
            nc.vector.memset(dxin, 0.0)

            # xbar: dummy transpose absorbs the copy->transpose mode switch
            dummy_inst = nc.sync.dma_start_transpose(out=dxout, in_=dxin)

            # ---- ACT preamble: cast + squares + stat scalars ----
            nc.scalar.activation(aib, ai, AF.Copy)
            nc.scalar.activation(sqj[:, 0:8, :], cj[:, 0:8, :], AF.Square)
            nc.scalar.activation(sqj[:, 8:16, :], cj[:, 8:16, :], AF.Square)

            # ---- DVE preamble ----
            nc.vector.tensor_mul(sqi, aib, aib)               # 2x (bf16)
            nc.vector.tensor_reduce(out=n2c[:, 0:8], in_=sqj[:, 0:8, :],
                                    axis=AX.X, op=ALU.add)
            nc.vector.tensor_reduce(out=n2i, in_=sqi, axis=AX.X, op=ALU.add)

            lgc1 = scratch.tile([P, 8], f32, tag="lgc1")
            nc.scalar.activation(lgc1, n2c[:, 0:8], AF.Ln, bias=zb)
            nc.scalar.activation(inv4c[:, 0:8, 0], lgc1, AF.Exp,
                                 scale=-0.5, bias=b_ln4)

            a_ap, b_ap = broadcast_tensor_aps(cj[:, 0:8, :], inv4c[:, 0:8, :])
            nc.vector.tensor_tensor(out=zcj[:, 0:8, :], in0=a_ap, in1=b_ap,
                                    op=ALU.mult)

            nc.vector.tensor_reduce(out=n2c[:, 8:16], in_=sqj[:, 8:16, :],
                                    axis=AX.X, op=ALU.add)

            lgc2 = scratch.tile([P, 8], f32, tag="lgc2")
            nc.scalar.activation(lgc2, n2c[:, 8:16], AF.Ln, bias=zb)
            nc.scalar.activation(inv4c[:, 8:16, 0], lgc2, AF.Exp,
                                 scale=-0.5, bias=b_ln4)
            lgi = scratch.tile([P, RT], f32, tag="lgi")
            nc.scalar.activation(lgi, n2i, AF.Ln, bias=zb)
            nc.scalar.activation(invi, lgi, AF.Exp, scale=-0.5, bias=zb)

            a_ap, b_ap = broadcast_tensor_aps(cj[:, 8:16, :], inv4c[:, 8:16, :])
            nc.vector.tensor_tensor(out=zcj[:, 8:16, :], in0=a_ap, in1=b_ap,
                                    op=ALU.mult)

            # ---- transposes (xbar serial): aibT, then zcjT halves ----
            t1 = nc.sync.dma_start_transpose(out=aibT, in_=aib)
            add_dep_helper(t1.ins, dummy_inst.ins, False, "xpose after dummy")
            t2 = nc.sync.dma_start_transpose(out=zcjT[:, 0:8, :],
                                             in_=zcj[:, 0:8, :])
            add_dep_helper(t2.ins, dummy_inst.ins, False, "xpose after dummy")
            t3 = nc.sync.dma_start_transpose(out=zcjT[:, 8:16, :],
                                             in_=zcj[:, 8:16, :])
            add_dep_helper(t3.ins, dummy_inst.ins, False, "xpose after dummy")

            # ---- diag elementwise on GPSIMD (reduces later on DVE) ----
            sqo = scratch.tile([P, RT, D], bf16, tag="sqo")
            nc.gpsimd.tensor_mul(sqo, oj, oj)
            ddt = scratch.tile([P, RT, D], bf16, tag="ddt")
            nc.gpsimd.tensor_mul(ddt, aib, oj)

            zcjT_flat = zcjT.rearrange("p t d -> p (t d)")
            aibT_flat = aibT.rearrange("p t d -> p (t d)")

            # ---- main loop: 8 chunks (one row tile each) ----
            eb_last = None
            for k in range(RT):
                ps = psmain.tile([P, CB], f32, tag="ps")
                for q in range(4):
                    nc.tensor.matmul(
                        ps[:, q * 512:(q + 1) * 512],
                        aibT[:, k, :],
                        zcjT_flat[:, q * 512:(q + 1) * 512],
                        start=True,
                        stop=True,
                    )
                eb = ebuf.tile([P, CB], bf16, tag="eb")
                eb_last = eb
                nc.scalar.activation(
                    eb, ps, AF.Exp,
                    scale=invi[:, k:k + 1],
                    bias=zb,
                    accum_out=rs_sb[:, k:k + 1],
                )
                if k == 0:
                    nc.vector.tensor_copy(esum[0], eb)        # 4x
                else:
                    nc.vector.tensor_tensor(
                        out=esum[k % 2], in0=eb, in1=esum[(k + 1) % 2],
                        op=ALU.add,
                    )                                          # 2x
                if k == 2:
                    # squeeze diag reduces into DVE slack mid-loop
                    nc.vector.tensor_reduce(out=n2o, in_=sqo, axis=AX.X,
                                            op=ALU.add)
                if k == 4:
                    nc.vector.tensor_reduce(out=rvec, in_=ddt, axis=AX.X,
                                            op=ALU.add)

            es_fin = esum[(RT - 1) % 2]

            # ---- tail: colsum partials via mask matmuls ----
            psR_full = psmain.tile([P, CB], f32, tag="ps")
            for q in range(4):
                nc.tensor.matmul(
                    psR_full[0:2, q * 512:(q + 1) * 512],
                    mask2,
                    es_fin[:, q * 512:(q + 1) * 512],
                    start=True,
                    stop=True,
                )
            nc.scalar.activation(cs_sb, psR_full[0:2, :], AF.Copy)

            nc.sync.dma_start(out=out_rs[:, :], in_=rs_sb)
            nc.sync.dma_start(out=out_cs[:, :], in_=cs_sb)
            nc.sync.dma_start(out=out_dg[:, :], in_=dg)

    return nc


def _split_multi_waits(bir: bytes) -> bytes:
    """The walrus build in this container accepts only ONE sync-wait per
    compute/DMA instruction. Tile emits up to three. Move all but one wait
    onto standalone EventSemaphore instructions inserted just before the
    offender on the same engine queue."""
    import json

    d = json.loads(bir)
    n_split = 0
    for fn in d["functions"]:
        for blk in fn["blocks"]:
            new_insts = []
            for ins in blk["instructions"]:
                si = ins.get("sync_info")
                waits = (si or {}).get("on_wait") or []
                if len(waits) > 1:
                    for w in waits[:-1]:
                        ev = {
                            "debug": ins.get("debug", 0),
                            "engine": ins["engine"],
                            "ins": [],
                            "outs": [],
                            "name": f"{ins['name']}_wsplit{n_split}",
                            "opcode": "EventSemaphore",
                            "sync_info": {"on_update": [], "on_wait": [w]},
                        }
                        n_split += 1
                        new_insts.append(ev)
                    si["on_wait"] = [waits[-1]]
                new_insts.append(ins)
            blk["instructions"] = new_insts
    return json.dumps(d).encode()


def kernel(emb_i: np.ndarray, emb_j: np.ndarray) -> np.ndarray:
    from concourse.bass_utils import run_bass_kernel_spmd

    if "nc" not in _cache:
        nc = _build_bass()
        fixed = _split_multi_waits(nc.to_json_bytes())
        nc.to_json_bytes = lambda: fixed
        _cache["nc"] = nc
    nc = _cache["nc"]

    emb_i = np.ascontiguousarray(emb_i, dtype=np.float32)
    emb_j = np.ascontiguousarray(emb_j, dtype=np.float32)
    in_maps = []
    for c in range(NCORES):
        rb, ch = c // 2, c % 2
        in_maps.append(
            {
                "emb_i_blk": emb_i[rb * RB:(rb + 1) * RB],
                "emb_j_cols": emb_j[ch * CB:(ch + 1) * CB],
                "emb_j_own": emb_j[rb * RB:(rb + 1) * RB],
            }
        )

    import os

    trace = bool(os.environ.get("KERNEL_TRACE"))
    res = run_bass_kernel_spmd(
        nc, in_maps, core_ids=list(range(NCORES)), trace=trace
    )
    _cache["last_res"] = res

    # ---- host combine ----
    rs_total = np.zeros(B, dtype=np.float64)
    cs_total = np.zeros(B, dtype=np.float64)
    dtot = np.float64(0.0)
    for c, r in enumerate(res.results):
        rb, ch = c // 2, c % 2
        # rowsum [128, 8]: (p, k) -> local row p*8+k
        rs_total[rb * RB:(rb + 1) * RB] += (
            r["rowsum"].astype(np.float64).reshape(RB)
        )
        # colsum [2, 2048]: both mask cols are all-ones, so the two
        # partition rows are identical copies -> average them.
        csv = r["colsum"].astype(np.float64).mean(axis=0)
        cs_total[ch * CB:(ch + 1) * CB] += (
            csv.reshape(CT, P).T.reshape(CB)
        )
        if ch == 0:
            d = r["diag"].astype(np.float64)
            rv = d[:, 0:RT].reshape(RB)
            no = d[:, RT:2 * RT].reshape(RB)
            iv = d[:, 2 * RT:3 * RT].reshape(RB)
            # pos/T = 4 * rvec * invi * 1/sqrt(n2o); contributes -2*pos/T
            dtot += np.sum(-8.0 * rv * iv / np.sqrt(no))
    total = dtot + np.log(rs_total).sum() + np.log(cs_total).sum()
    loss = total / (2 * B)
    return np.array(loss, dtype=np.float32)


# revision 13
# speedup vs baseline: 1.1868x; 1.0019x over previous
"""Trainium2 Bass kernel for NT-Xent style contrastive loss (v2).

Math (B=4096, D=128, T=0.25), z = row-normalized emb:
  S = z_i @ z_j^T   [B, B]
  loss = (1/2B) * sum_r [ -2*S[r,r]/T + ln(sum_c exp(S[r,c]/T))
                                      + ln(sum_c exp(S[c,r]/T)) ]

Sharding: core (rb, ch), rb = core//2, ch = core%2: rows rb*1024 of emb_i,
cols ch*2048 of emb_j. All loads are HWDGE fp32 with "(p t) d" tiling so
each DMA descriptor is 2-4KB contiguous (row g of a block sits at
partition g//T, tile g%T; host unpermutes the tiny outputs).

Orientation: ps chunk k = [128 r (row-tile k), 2048 c free], 8 chunks.
  stationary = aibT tile k  (RAW bf16-cast emb_i, transposed — row stats
               are NOT on the matmul critical path)
  moving     = zcjT [d, 2048] (cols pre-scaled by 4/|c| — absorbs 1/T)
The EXP applies the row norm via its per-partition scale AP
(scale = invi[:,k], 1/|a_r|) and its accumulator emits the ROW SUMS
directly (accum_out -> rs[:, k]).  Column-sum partials come from a
bf16 TT esum chain (DVE 2x mode, one op per chunk) reduced over
partitions by 4 tail mask-matmuls -> [2, 2048] PSUM.

Engine split:
  ACT: ai cast, cj squares, ln/exp stat scalars, 8 big EXP+accum, cs copy
  DVE: sq-i (2x), 3 free-axis reduces + 2 col scales (preamble),
       esum copy+TT chain (2x/4x), diag reduces squeezed mid-loop
  GP : diag elementwise mults (sq-o, ddt) — GP cannot reduce free axis
  PE : 32 main matmuls N=512 + 4 tail mask-matmuls
  XBAR: dummy + aibT + zcjT in 2 halves

Host: unpermute + sum partials, ln, diag terms, /2B.
"""

import numpy as np

B = 4096
D = 128
P = 128
NCORES = 8
RB = 1024                  # rows per core
CB = 2048                  # cols per core
RT = RB // P               # 8 row tiles
CT = CB // P               # 16 col tiles
TEMP = 0.25
LN4 = float(np.log(4.0))

_cache = {}


def _build_bass():
    import concourse.bass as bass
    import concourse.mybir as mybir
    import concourse.tile as tile
    from concourse.bass import broadcast_tensor_aps
    from concourse.tile_rust import add_dep_helper

    f32 = mybir.dt.float32
    bf16 = mybir.dt.bfloat16
    AF = mybir.ActivationFunctionType
    ALU = mybir.AluOpType
    AX = mybir.AxisListType

    nc = bass.Bass("TRN2")
    ai_d = nc.dram_tensor("emb_i_blk", [RB, D], f32, kind="ExternalInput")
    cj_d = nc.dram_tensor("emb_j_cols", [CB, D], f32, kind="ExternalInput")
    oj_d = nc.dram_tensor("emb_j_own", [RB, D], f32, kind="ExternalInput")
    out_rs = nc.dram_tensor("rowsum", [P, RT], f32, kind="ExternalOutput")
    out_cs = nc.dram_tensor("colsum", [P, 512], f32, kind="ExternalOutput")
    out_dg = nc.dram_tensor("diag", [P, 3 * RT], f32, kind="ExternalOutput")

    # (p t) d: partition p holds rows p*T .. p*T+T-1 -> 2-4KB descriptors
    ai_t = ai_d.rearrange("(p t) d -> p t d", p=P)   # row g = p*8 + t
    cj_t = cj_d.rearrange("(p t) d -> p t d", p=P)   # col g = p*16 + t
    oj_t = oj_d.rearrange("(p t) d -> p t d", p=P)

    with tile.TileContext(nc) as tc:
        with (
            tc.tile_pool(name="persist", bufs=1) as persist,
            tc.tile_pool(name="scratch", bufs=4) as scratch,
            tc.tile_pool(name="ebuf", bufs=2) as ebuf,
            tc.tile_pool(name="psmain", bufs=2, space="PSUM") as psmain,
        ):
            ai = persist.tile([P, RT, D], f32, tag="ai")
            cj = persist.tile([P, CT, D], f32, tag="cj")
            oj = persist.tile([P, RT, D], f32, tag="oj")
            aib = persist.tile([P, RT, D], bf16, tag="aib")
            aibT = persist.tile([P, RT, D], bf16, tag="aibT")
            sqj = persist.tile([P, CT, D], bf16, tag="sqj")
            sqi = persist.tile([P, RT, D], bf16, tag="sqi")
            zcj = persist.tile([P, CT, D], bf16, tag="zcj")
            zcjT = persist.tile([P, CT, D], bf16, tag="zcjT")
            n2c = persist.tile([P, CT], f32, tag="n2c")
            n2i = persist.tile([P, RT], f32, tag="n2i")
            inv4c = persist.tile([P, CT, 1], bf16, tag="inv4c")
            dg = persist.tile([P, 3 * RT], f32, tag="dg")  # rvec|n2o|invi
            rs_sb = persist.tile([P, RT], f32, tag="rs_sb")
            cs_sb = persist.tile([P, 512], f32, tag="cs_sb")
            esum = [
                persist.tile([P, CB], bf16, name="esum0", tag="esum0"),
                persist.tile([P, CB], bf16, name="esum1", tag="esum1"),
            ]
            zb = persist.tile([P, 1], f32, tag="zb")
            b_ln4 = persist.tile([P, 1], f32, tag="b_ln4")
            mask2 = persist.tile([P, 2], bf16, tag="mask2")
            dxin = persist.tile([16, D], bf16, tag="dxin")
            dxout = persist.tile([P, 16], bf16, tag="dxout")

            rvec = dg[:, 0:RT]
            n2o = dg[:, RT:2 * RT]
            invi = dg[:, 2 * RT:3 * RT]

            # ---- loads (HWDGE fp32, big descriptors); cj first: its
            # stats->scale->transpose chain is the critical path ----
            nc.sync.dma_start(out=cj[:, 0:8, :], in_=cj_t[:, 0:8, :])
            nc.sync.dma_start(out=cj[:, 8:16, :], in_=cj_t[:, 8:16, :])
            nc.sync.dma_start(out=ai, in_=ai_t)
            nc.sync.dma_start(out=oj, in_=oj_t)

            # ---- tiny constants ----
            nc.vector.memset(zb, 0.0)
            nc.vector.memset(b_ln4, LN4)
            nc.vector.memset(mask2, 1.0)
            nc.vector.memset(dxin, 0.0)

            # xbar: dummy transpose absorbs the copy->transpose mode switch
            dummy_inst = nc.sync.dma_start_transpose(out=dxout, in_=dxin)

            # ---- ACT preamble: squares + cast + stat scalars ----
            nc.scalar.activation(sqj[:, 0:8, :], cj[:, 0:8, :], AF.Square)
            nc.scalar.activation(sqj[:, 8:16, :], cj[:, 8:16, :], AF.Square)

            # ---- DVE preamble (order = queue order) ----
            nc.vector.tensor_reduce(out=n2c[:, 0:8], in_=sqj[:, 0:8, :],
                                    axis=AX.X, op=ALU.add)

            lgc1 = scratch.tile([P, 8], f32, tag="lgc1")
            nc.scalar.activation(lgc1, n2c[:, 0:8], AF.Ln, bias=zb)
            nc.scalar.activation(inv4c[:, 0:8, 0], lgc1, AF.Exp,
                                 scale=-0.5, bias=b_ln4)
            nc.scalar.activation(aib, ai, AF.Copy)
            nc.scalar.activation(sqi, ai, AF.Square)

            nc.vector.tensor_reduce(out=n2c[:, 8:16], in_=sqj[:, 8:16, :],
                                    axis=AX.X, op=ALU.add)

            a_ap, b_ap = broadcast_tensor_aps(cj[:, 0:8, :], inv4c[:, 0:8, :])
            nc.vector.tensor_tensor(out=zcj[:, 0:8, :], in0=a_ap, in1=b_ap,
                                    op=ALU.mult)

            lgc2 = scratch.tile([P, 8], f32, tag="lgc2")
            nc.scalar.activation(lgc2, n2c[:, 8:16], AF.Ln, bias=zb)
            nc.scalar.activation(inv4c[:, 8:16, 0], lgc2, AF.Exp,
                                 scale=-0.5, bias=b_ln4)

            nc.vector.tensor_reduce(out=n2i, in_=sqi, axis=AX.X, op=ALU.add)

            a_ap, b_ap = broadcast_tensor_aps(cj[:, 8:16, :], inv4c[:, 8:16, :])
            nc.vector.tensor_tensor(out=zcj[:, 8:16, :], in0=a_ap, in1=b_ap,
                                    op=ALU.mult)

            lgi = scratch.tile([P, RT], f32, tag="lgi")
            nc.scalar.activation(lgi, n2i, AF.Ln, bias=zb)
            nc.scalar.activation(invi, lgi, AF.Exp, scale=-0.5, bias=zb)

            # ---- transposes (xbar serial): aibT, then zcjT halves ----
            t1 = nc.sync.dma_start_transpose(out=aibT, in_=aib)
            add_dep_helper(t1.ins, dummy_inst.ins, False, "xpose after dummy")
            t2 = nc.sync.dma_start_transpose(out=zcjT[:, 0:8, :],
                                             in_=zcj[:, 0:8, :])
            add_dep_helper(t2.ins, dummy_inst.ins, False, "xpose after dummy")
            t3 = nc.sync.dma_start_transpose(out=zcjT[:, 8:16, :],
                                             in_=zcj[:, 8:16, :])
            add_dep_helper(t3.ins, dummy_inst.ins, False, "xpose after dummy")

            # ---- diag elementwise on GPSIMD (reduces later on DVE) ----
            sqo = scratch.tile([P, RT, D], bf16, tag="sqo")
            nc.gpsimd.tensor_mul(sqo, oj, oj)
            ddt = scratch.tile([P, RT, D], bf16, tag="ddt")
            nc.gpsimd.tensor_mul(ddt, aib, oj)

            zcjT_flat = zcjT.rearrange("p t d -> p (t d)")
            aibT_flat = aibT.rearrange("p t d -> p (t d)")

            # ---- main loop: 8 chunks (one row tile each) ----
            eb_last = None
            for k in range(RT):
                ps = psmain.tile([P, CB], f32, tag="ps")
                for q in range(4):
                    nc.tensor.matmul(
                        ps[:, q * 512:(q + 1) * 512],
                        aibT[:, k, :],
                        zcjT_flat[:, q * 512:(q + 1) * 512],
                        start=True,
                        stop=True,
                    )
                eb = ebuf.tile([P, CB], bf16, tag="eb")
                eb_last = eb
                nc.scalar.activation(
                    eb, ps, AF.Exp,
                    scale=invi[:, k:k + 1],
                    bias=zb,
                    accum_out=rs_sb[:, k:k + 1],
                )
                if k == 0:
                    nc.vector.tensor_copy(esum[0], eb)        # 4x
                elif k < RT - 1:
                    nc.vector.tensor_tensor(
                        out=esum[k % 2], in0=eb, in1=esum[(k + 1) % 2],
                        op=ALU.add,
                    )                                          # 2x
                if k == 2:
                    # squeeze diag reduces into DVE slack mid-loop
                    nc.vector.tensor_reduce(out=n2o, in_=sqo, axis=AX.X,
                                            op=ALU.add)
                if k == 4:
                    nc.vector.tensor_reduce(out=rvec, in_=ddt, axis=AX.X,
                                            op=ALU.add)

            es_fin = esum[(RT - 2) % 2]     # chain through chunk 6

            # ---- tail: colsum partials via accumulating mask matmuls:
            # group 1 over esum(0..6) can start before the last EXP;
            # group 2 adds chunk 7's eb directly. ----
            # q-th block lands on PSUM partitions 32q..32q+1, free 0:512,
            # so one free-size-512 ACT copy extracts all four.
            psR_full = psmain.tile([P, CB], f32, tag="ps")
            for q in range(4):
                nc.tensor.matmul(
                    psR_full[32 * q:32 * q + 2, 0:512],
                    mask2,
                    es_fin[:, q * 512:(q + 1) * 512],
                    start=True,
                    stop=False,
                    tile_position=(0, 32 * q),
                )
            for q in range(4):
                nc.tensor.matmul(
                    psR_full[32 * q:32 * q + 2, 0:512],
                    mask2,
                    eb_last[:, q * 512:(q + 1) * 512],
                    start=False,
                    stop=True,
                    tile_position=(0, 32 * q),
                )
            nc.scalar.activation(cs_sb, psR_full[:, 0:512], AF.Copy)

            nc.sync.dma_start(out=out_dg[:, :], in_=dg)
            nc.sync.dma_start(out=out_rs[:, :], in_=rs_sb)
            nc.sync.dma_start(out=out_cs[:, :], in_=cs_sb)

    return nc


def _split_multi_waits(bir: bytes) -> bytes:
    """The walrus build in this container accepts only ONE sync-wait per
    compute/DMA instruction. Tile emits up to three. Move all but one wait
    onto standalone EventSemaphore instructions inserted just before the
    offender on the same engine queue."""
    import json

    d = json.loads(bir)
    n_split = 0
    for fn in d["functions"]:
        for blk in fn["blocks"]:
            new_insts = []
            for ins in blk["instructions"]:
                si = ins.get("sync_info")
                waits = (si or {}).get("on_wait") or []
                if len(waits) > 1:
                    for w in waits[:-1]:
                        ev = {
                            "debug": ins.get("debug", 0),
                            "engine": ins["engine"],
                            "ins": [],
                            "outs": [],
                            "name": f"{ins['name']}_wsplit{n_split}",
                            "opcode": "EventSemaphore",
                            "sync_info": {"on_update": [], "on_wait": [w]},
                        }
                        n_split += 1
                        new_insts.append(ev)
                    si["on_wait"] = [waits[-1]]
                new_insts.append(ins)
            blk["instructions"] = new_insts
    return json.dumps(d).encode()


def kernel(emb_i: np.ndarray, emb_j: np.ndarray) -> np.ndarray:
    from concourse.bass_utils import run_bass_kernel_spmd

    if "nc" not in _cache:
        nc = _build_bass()
        fixed = _split_multi_waits(nc.to_json_bytes())
        nc.to_json_bytes = lambda: fixed
        _cache["nc"] = nc
    nc = _cache["nc"]

    emb_i = np.ascontiguousarray(emb_i, dtype=np.float32)
    emb_j = np.ascontiguousarray(emb_j, dtype=np.float32)
    in_maps = []
    for c in range(NCORES):
        rb, ch = c // 2, c % 2
        in_maps.append(
            {
                "emb_i_blk": emb_i[rb * RB:(rb + 1) * RB],
                "emb_j_cols": emb_j[ch * CB:(ch + 1) * CB],
                "emb_j_own": emb_j[rb * RB:(rb + 1) * RB],
            }
        )

    import os

    trace = bool(os.environ.get("KERNEL_TRACE"))
    res = run_bass_kernel_spmd(
        nc, in_maps, core_ids=list(range(NCORES)), trace=trace
    )
    _cache["last_res"] = res

    # ---- host combine ----
    rs_total = np.zeros(B, dtype=np.float64)
    cs_total = np.zeros(B, dtype=np.float64)
    dtot = np.float64(0.0)
    for c, r in enumerate(res.results):
        rb, ch = c // 2, c % 2
        # rowsum [128, 8]: (p, k) -> local row p*8+k
        rs_total[rb * RB:(rb + 1) * RB] += (
            r["rowsum"].astype(np.float64).reshape(RB)
        )
        # colsum [128, 512]: q-th 512-block of the free axis lives on
        # partitions 32q..32q+1 (two identical all-ones mask rows).
        co = r["colsum"].astype(np.float64)
        csv = np.concatenate(
            [0.5 * (co[32 * q] + co[32 * q + 1]) for q in range(4)]
        )
        cs_total[ch * CB:(ch + 1) * CB] += (
            csv.reshape(CT, P).T.reshape(CB)
        )
        if ch == 0:
            d = r["diag"].astype(np.float64)
            rv = d[:, 0:RT].reshape(RB)
            no = d[:, RT:2 * RT].reshape(RB)
            iv = d[:, 2 * RT:3 * RT].reshape(RB)
            # pos/T = 4 * rvec * invi * 1/sqrt(n2o); contributes -2*pos/T
            dtot += np.sum(-8.0 * rv * iv / np.sqrt(no))
    total = dtot + np.log(rs_total).sum() + np.log(cs_total).sum()
    loss = total / (2 * B)
    return np.array(loss, dtype=np.float32)


# revision 17
# speedup vs baseline: 1.2664x; 1.0671x over previous
"""Trainium2 Bass kernel for NT-Xent style contrastive loss (v4).

Math (B=4096, D=128, T=0.25), z = row-normalized emb:
  S = z_i @ z_j^T   [B, B]
  loss = (1/2B) * sum_r [ -2*S[r,r]/T + ln(sum_c exp(S[r,c]/T))
                                      + ln(sum_c exp(S[c,r]/T)) ]

Sharding: core (rb, ch), rb = core//2, ch = core%2: rows rb*1024 of emb_i,
cols ch*2048 of emb_j.  All loads are HWDGE fp32 with "(p t) d" 8-row
tiling (partition g//8, tile g%8 -> 2-4KB contiguous descriptors; the tiny
outputs are unpermuted on the host).  emb_j's block is loaded as two
1024-row halves with the SAME tiling as emb_i's block, so the diagonal
dot products are computed against cj directly (each row block equals one
cj half on cores 0/2/5/7) and no separate "own rows" load is needed;
the column norms n2c double as the diagonal's |z_j| norms.

Orientation: ps chunk k = [128 r (row-tile k), 2048 c free], 8 chunks.
  stationary = aibT tile k (RAW bf16-cast emb_i, transposed; row stats
               are off the matmul critical path)
  moving     = zcjT quarter tiles (cols pre-scaled by 4/|c|, absorbing
               1/T), one per N=512 matmul
The EXP applies the row norm via its per-partition scale AP
(scale = invi[:,k]) and its accumulator emits the ROW SUMS directly
(accum_out -> rs[:, k]).  Column-sum partials: bf16 TT esum chain
(DVE 2x, chunks 1-6) + accumulating mask matmuls over {esum, eb_7},
stacked on PSUM partitions 32q so one free-512 ACT copy extracts them.

The cj pipeline is quarter-granular (sq -> reduce -> ln/exp -> scale ->
transpose per 256KB quarter) so compute starts while later quarters are
still on the wire; loads issue from four different engine DGE queues to
overlap issue latency.
"""

import numpy as np

B = 4096
D = 128
P = 128
NCORES = 8
RB = 1024                  # rows per core
CB = 2048                  # cols per core
RT = RB // P               # 8 row tiles
HT = 8                     # tiles per cj half (8-row tiling)
TEMP = 0.25
LN4 = float(np.log(4.0))

_cache = {}


def _build_bass():
    import concourse.bass as bass
    import concourse.mybir as mybir
    import concourse.tile as tile
    from concourse.bass import broadcast_tensor_aps
    from concourse.tile_rust import add_dep_helper

    f32 = mybir.dt.float32
    bf16 = mybir.dt.bfloat16
    AF = mybir.ActivationFunctionType
    ALU = mybir.AluOpType
    AX = mybir.AxisListType

    nc = bass.Bass("TRN2")
    ai_d = nc.dram_tensor("emb_i_blk", [RB, D], f32, kind="ExternalInput")
    ca_d = nc.dram_tensor("emb_j_ca", [RB, D], f32, kind="ExternalInput")
    cb_d = nc.dram_tensor("emb_j_cb", [RB, D], f32, kind="ExternalInput")
    out_rs = nc.dram_tensor("rowsum", [P, RT], f32, kind="ExternalOutput")
    out_cs = nc.dram_tensor("colsum", [P, 512], f32, kind="ExternalOutput")
    out_dg = nc.dram_tensor("diag", [P, 5 * RT], f32, kind="ExternalOutput")

    ai_t = ai_d.rearrange("(p t) d -> p t d", p=P)   # row g = p*8 + t
    ca_t = ca_d.rearrange("(p t) d -> p t d", p=P)   # col g = p*8 + t
    cb_t = cb_d.rearrange("(p t) d -> p t d", p=P)   # col g = 1024 + p*8 + t

    with tile.TileContext(nc) as tc:
        with (
            tc.tile_pool(name="persist", bufs=1) as persist,
            tc.tile_pool(name="scratch", bufs=4) as scratch,
            tc.tile_pool(name="ebuf", bufs=2) as ebuf,
            tc.tile_pool(name="psmain", bufs=2, space="PSUM") as psmain,
        ):
            ai = persist.tile([P, RT, D], f32, tag="ai")
            # cj quarters: [half a tiles 0-3, a 4-7, b 0-3, b 4-7]
            cjq = [persist.tile([P, 4, D], f32, name=f"cjq{q}", tag=f"cjq{q}")
                   for q in range(4)]
            aib = persist.tile([P, RT, D], bf16, tag="aib")
            aibT = persist.tile([P, RT, D], bf16, tag="aibT")
            sqi = persist.tile([P, RT, D], bf16, tag="sqi")
            sqq = [persist.tile([P, 4, D], bf16, name=f"sqq{q}", tag=f"sqq{q}")
                   for q in range(4)]
            zcq = [persist.tile([P, 4, D], bf16, name=f"zcq{q}", tag=f"zcq{q}")
                   for q in range(4)]
            zcqT = [persist.tile([P, 4, D], bf16, name=f"zcqT{q}",
                                 tag=f"zcqT{q}") for q in range(4)]
            n2c = persist.tile([P, 4, 4], f32, tag="n2c")    # [q][tile]
            n2i = persist.tile([P, RT], f32, tag="n2i")
            inv4c = persist.tile([P, 4, 4, 1], bf16, tag="inv4c")
            dg = persist.tile([P, 5 * RT], f32, tag="dg")
            rs_sb = persist.tile([P, RT], f32, tag="rs_sb")
            cs_sb = persist.tile([P, 512], f32, tag="cs_sb")
            esum = [
                persist.tile([P, CB], bf16, name="esum0", tag="esum0"),
                persist.tile([P, CB], bf16, name="esum1", tag="esum1"),
            ]
            zb = persist.tile([P, 1], f32, tag="zb")
            b_ln4 = persist.tile([P, 1], f32, tag="b_ln4")
            mask2 = persist.tile([P, 2], bf16, tag="mask2")
            dxin = persist.tile([16, D], bf16, tag="dxin")
            dxout = persist.tile([P, 16], bf16, tag="dxout")

            rva = dg[:, 0:RT]
            rvb = dg[:, RT:2 * RT]
            invi = dg[:, 2 * RT:3 * RT]
            n2c_out = dg[:, 3 * RT:5 * RT]       # n2c copy [p, 16]

            # ---- loads: ai + quarters, issued on 4 different DGE
            # queues so issue latency overlaps; wire-time is shared ----
            nc.sync.dma_start(out=ai, in_=ai_t)
            nc.scalar.dma_start(out=cjq[0], in_=ca_t[:, 0:4, :])
            nc.sync.dma_start(out=cjq[1], in_=ca_t[:, 4:8, :])
            nc.gpsimd.dma_start(out=cjq[2], in_=cb_t[:, 0:4, :])
            nc.scalar.dma_start(out=cjq[3], in_=cb_t[:, 4:8, :])

            # ---- tiny constants ----
            nc.vector.memset(zb, 0.0)
            nc.vector.memset(b_ln4, LN4)
            nc.vector.memset(mask2, 1.0)
            nc.vector.memset(dxin, 0.0)

            dummy_inst = nc.sync.dma_start_transpose(out=dxout, in_=dxin)

            # ---- ACT: cast ai, square q1/q2, stat scalars asap ----
            nc.scalar.activation(aib, ai, AF.Copy)
            nc.scalar.activation(sqq[0], cjq[0], AF.Square)
            nc.scalar.activation(sqq[1], cjq[1], AF.Square)
            nc.scalar.activation(sqq[2], cjq[2], AF.Square)

            # ---- DVE: row stats first (invi gates only the EXPs) ----
            nc.vector.tensor_mul(sqi, aib, aib)               # 2x bf16
            nc.vector.tensor_reduce(out=n2i, in_=sqi, axis=AX.X, op=ALU.add)
            nc.vector.tensor_reduce(out=n2c[:, 0, :], in_=sqq[0],
                                    axis=AX.X, op=ALU.add)

            lgi = scratch.tile([P, RT], f32, tag="lgi")
            nc.scalar.activation(lgi, n2i, AF.Ln, bias=zb)
            nc.scalar.activation(invi, lgi, AF.Exp, scale=-0.5, bias=zb)
            lgq0 = scratch.tile([P, 4], f32, tag="lgq0")
            nc.scalar.activation(lgq0, n2c[:, 0, :], AF.Ln, bias=zb)
            nc.scalar.activation(inv4c[:, 0, :, 0], lgq0, AF.Exp,
                                 scale=-0.5, bias=b_ln4)

            nc.vector.tensor_reduce(out=n2c[:, 1, :], in_=sqq[1],
                                    axis=AX.X, op=ALU.add)
            a_ap, b_ap = broadcast_tensor_aps(cjq[0], inv4c[:, 0, :, :])
            nc.vector.tensor_tensor(out=zcq[0], in0=a_ap, in1=b_ap,
                                    op=ALU.mult)

            lgq1 = scratch.tile([P, 4], f32, tag="lgq1")
            nc.scalar.activation(lgq1, n2c[:, 1, :], AF.Ln, bias=zb)
            nc.scalar.activation(inv4c[:, 1, :, 0], lgq1, AF.Exp,
                                 scale=-0.5, bias=b_ln4)
            nc.scalar.activation(sqq[3], cjq[3], AF.Square)

            nc.vector.tensor_reduce(out=n2c[:, 2, :], in_=sqq[2],
                                    axis=AX.X, op=ALU.add)
            a_ap, b_ap = broadcast_tensor_aps(cjq[1], inv4c[:, 1, :, :])
            nc.vector.tensor_tensor(out=zcq[1], in0=a_ap, in1=b_ap,
                                    op=ALU.mult)

            lgq2 = scratch.tile([P, 4], f32, tag="lgq2")
            nc.scalar.activation(lgq2, n2c[:, 2, :], AF.Ln, bias=zb)
            nc.scalar.activation(inv4c[:, 2, :, 0], lgq2, AF.Exp,
                                 scale=-0.5, bias=b_ln4)

            nc.vector.tensor_reduce(out=n2c[:, 3, :], in_=sqq[3],
                                    axis=AX.X, op=ALU.add)
            a_ap, b_ap = broadcast_tensor_aps(cjq[2], inv4c[:, 2, :, :])
            nc.vector.tensor_tensor(out=zcq[2], in0=a_ap, in1=b_ap,
                                    op=ALU.mult)

            lgq3 = scratch.tile([P, 4], f32, tag="lgq3")
            nc.scalar.activation(lgq3, n2c[:, 3, :], AF.Ln, bias=zb)
            nc.scalar.activation(inv4c[:, 3, :, 0], lgq3, AF.Exp,
                                 scale=-0.5, bias=b_ln4)

            a_ap, b_ap = broadcast_tensor_aps(cjq[3], inv4c[:, 3, :, :])
            nc.vector.tensor_tensor(out=zcq[3], in0=a_ap, in1=b_ap,
                                    op=ALU.mult)

            # n2c ships to host (diagonal needs |z_j| of own rows)
            nc.vector.tensor_copy(n2c_out, n2c.rearrange("p a b -> p (a b)"))

            # ---- transposes (xbar serial): aibT early, quarters asap ----
            t1 = nc.sync.dma_start_transpose(out=aibT, in_=aib)
            add_dep_helper(t1.ins, dummy_inst.ins, False, "xpose after dummy")
            for q in range(4):
                tq = nc.sync.dma_start_transpose(out=zcqT[q], in_=zcq[q])
                add_dep_helper(tq.ins, dummy_inst.ins, False,
                               "xpose after dummy")

            # ---- diag elementwise on GPSIMD ----
            dda = scratch.tile([P, RT, D], bf16, tag="dda")
            ddb = scratch.tile([P, RT, D], bf16, tag="ddb")
            # half a = quarters 0,1 ; half b = quarters 2,3
            nc.gpsimd.tensor_mul(dda[:, 0:4, :], aib[:, 0:4, :], cjq[0])
            nc.gpsimd.tensor_mul(dda[:, 4:8, :], aib[:, 4:8, :], cjq[1])
            nc.gpsimd.tensor_mul(ddb[:, 0:4, :], aib[:, 0:4, :], cjq[2])
            nc.gpsimd.tensor_mul(ddb[:, 4:8, :], aib[:, 4:8, :], cjq[3])

            # ---- main loop: 8 chunks (one row tile each) ----
            eb_last = None
            for k in range(RT):
                ps = psmain.tile([P, CB], f32, tag="ps")
                for q in range(4):
                    nc.tensor.matmul(
                        ps[:, q * 512:(q + 1) * 512],
                        aibT[:, k, :],
                        zcqT[q].rearrange("p t d -> p (t d)"),
                        start=True,
                        stop=True,
                    )
                eb = ebuf.tile([P, CB], bf16, tag="eb")
                eb_last = eb
                nc.scalar.activation(
                    eb, ps, AF.Exp,
                    scale=invi[:, k:k + 1],
                    bias=zb,
                    accum_out=rs_sb[:, k:k + 1],
                )
                if k == 0:
                    nc.vector.tensor_copy(esum[0], eb)        # 4x
                elif k < RT - 1:
                    nc.vector.tensor_tensor(
                        out=esum[k % 2], in0=eb, in1=esum[(k + 1) % 2],
                        op=ALU.add,
                    )                                          # 2x
                if k == 2:
                    nc.vector.tensor_reduce(out=rva, in_=dda, axis=AX.X,
                                            op=ALU.add)
                if k == 4:
                    nc.vector.tensor_reduce(out=rvb, in_=ddb, axis=AX.X,
                                            op=ALU.add)

            es_fin = esum[(RT - 2) % 2]     # chain through chunk 6

            # ---- tail: colsum partials via accumulating mask matmuls;
            # q-th block on PSUM partitions 32q, free 0:512 ----
            psR_full = psmain.tile([P, CB], f32, tag="ps")
            for q in range(4):
                nc.tensor.matmul(
                    psR_full[32 * q:32 * q + 2, 0:512],
                    mask2,
                    es_fin[:, q * 512:(q + 1) * 512],
                    start=True,
                    stop=False,
                    tile_position=(0, 32 * q),
                )
            for q in range(4):
                nc.tensor.matmul(
                    psR_full[32 * q:32 * q + 2, 0:512],
                    mask2,
                    eb_last[:, q * 512:(q + 1) * 512],
                    start=False,
                    stop=True,
                    tile_position=(0, 32 * q),
                )
            nc.scalar.activation(cs_sb, psR_full[:, 0:512], AF.Copy)

            nc.sync.dma_start(out=out_dg[:, :], in_=dg)
            nc.sync.dma_start(out=out_rs[:, :], in_=rs_sb)
            nc.sync.dma_start(out=out_cs[:, :], in_=cs_sb)

    return nc


def _split_multi_waits(bir: bytes) -> bytes:
    """The walrus build in this container accepts only ONE sync-wait per
    compute/DMA instruction. Tile emits up to three. Move all but one wait
    onto standalone EventSemaphore instructions inserted just before the
    offender on the same engine queue."""
    import json

    d = json.loads(bir)
    n_split = 0
    for fn in d["functions"]:
        for blk in fn["blocks"]:
            new_insts = []
            for ins in blk["instructions"]:
                si = ins.get("sync_info")
                waits = (si or {}).get("on_wait") or []
                if len(waits) > 1:
                    for w in waits[:-1]:
                        ev = {
                            "debug": ins.get("debug", 0),
                            "engine": ins["engine"],
                            "ins": [],
                            "outs": [],
                            "name": f"{ins['name']}_wsplit{n_split}",
                            "opcode": "EventSemaphore",
                            "sync_info": {"on_update": [], "on_wait": [w]},
                        }
                        n_split += 1
                        new_insts.append(ev)
                    si["on_wait"] = [waits[-1]]
                new_insts.append(ins)
            blk["instructions"] = new_insts
    return json.dumps(d).encode()


def kernel(emb_i: np.ndarray, emb_j: np.ndarray) -> np.ndarray:
    from concourse.bass_utils import run_bass_kernel_spmd

    if "nc" not in _cache:
        nc = _build_bass()
        fixed = _split_multi_waits(nc.to_json_bytes())
        nc.to_json_bytes = lambda: fixed
        _cache["nc"] = nc
    nc = _cache["nc"]

    emb_i = np.ascontiguousarray(emb_i, dtype=np.float32)
    emb_j = np.ascontiguousarray(emb_j, dtype=np.float32)
    in_maps = []
    for c in range(NCORES):
        rb, ch = c // 2, c % 2
        in_maps.append(
            {
                "emb_i_blk": emb_i[rb * RB:(rb + 1) * RB],
                "emb_j_ca": emb_j[ch * CB:ch * CB + RB],
                "emb_j_cb": emb_j[ch * CB + RB:(ch + 1) * CB],
            }
        )

    import os

    trace = bool(os.environ.get("KERNEL_TRACE"))
    res = run_bass_kernel_spmd(
        nc, in_maps, core_ids=list(range(NCORES)), trace=trace
    )
    _cache["last_res"] = res

    # ---- host combine ----
    rs_total = np.zeros(B, dtype=np.float64)
    cs_total = np.zeros(B, dtype=np.float64)
    dtot = np.float64(0.0)
    for c, r in enumerate(res.results):
        rb, ch = c // 2, c % 2
        # rowsum [128, 8]: (p, k) -> local row p*8+k
        rs_total[rb * RB:(rb + 1) * RB] += (
            r["rowsum"].astype(np.float64).reshape(RB)
        )
        # colsum [128, 512]: q-th 512-block on partitions 32q..32q+1;
        # block free j = t*128 + p -> local col 1024*(q//2) + p*8
        #                            + 4*(q%2) + t
        co = r["colsum"].astype(np.float64)
        for q in range(4):
            blk = 0.5 * (co[32 * q] + co[32 * q + 1])        # [512]
            half, sub = q // 2, q % 2
            dst = cs_total[ch * CB + half * RB:ch * CB + half * RB + RB]
            dst.reshape(P, 2, 4)[:, sub, :] += blk.reshape(4, P).T
        if rb // 2 == ch:
            # this core's emb_i row block lies inside its cj col block
            d = r["diag"].astype(np.float64)
            iv = d[:, 2 * RT:3 * RT].reshape(RB)
            n2cc = d[:, 3 * RT:5 * RT]                       # [p, 16]
            half = rb % 2
            rv = d[:, half * RT:(half + 1) * RT].reshape(RB)
            # n2c [p, (q, tile)]: halves = q-pairs
            n2o = n2cc[:, half * RT:(half + 1) * RT].reshape(RB)
            # pos/T = 4 * rvec * invi / sqrt(n2o); contributes -2*pos/T
            dtot += np.sum(-8.0 * rv * iv / np.sqrt(n2o))
    total = dtot + np.log(rs_total).sum() + np.log(cs_total).sum()
    loss = total / (2 * B)
    return np.array(loss, dtype=np.float32)


# revision 18
# speedup vs baseline: 1.3164x; 1.0395x over previous
"""Trainium2 Bass kernel for NT-Xent style contrastive loss (v5).

Math (B=4096, D=128, T=0.25), z = row-normalized emb:
  S = z_i @ z_j^T   [B, B]
  loss = (1/2B) * sum_r [ -2*S[r,r]/T + ln(sum_c exp(S[r,c]/T))
                                      + ln(sum_c exp(S[c,r]/T)) ]

Sharding: core (rb, ch), rb = core//2, ch = core%2: rows rb*1024 of emb_i,
cols ch*2048 of emb_j.  Inputs are cast to bf16 ON THE HOST (input
quantization, ~0.4% per element, far inside the 2e-2 tolerance): halves
the DMA wire time, feeds the matmuls directly, and makes every stats op
2x-mode eligible.  All tensors use the "(p t) d" 8-row tiling (partition
g//8, tile g%8, 2KB contiguous descriptors); outputs are unpermuted on
the host.  emb_j's block is loaded as two 1024-row halves with the same
tiling as emb_i's block, so the diagonal dot products are computed
against cj directly (each row block equals one cj half on cores
0/2/5/7) and the column norms n2c double as the diagonal |z_j| norms.

Orientation: ps chunk k = [128 r (row-tile k), 2048 c free], 8 chunks.
  stationary = aibT tile k (RAW bf16 emb_i, transposed straight off the
               load; row stats are off the matmul critical path)
  moving     = zcjT halves (cols pre-scaled by 4/|c|, absorbing 1/T)
The EXP applies the row norm via its per-partition scale AP
(scale = invi[:,k]) and its accumulator emits the ROW SUMS directly
(accum_out -> rs[:, k]).  Column-sum partials: bf16 TT esum chain
(DVE 2x, chunks 1-6) + accumulating mask matmuls over {esum, eb_7},
stacked on PSUM partitions 32q so one free-512 ACT copy extracts them.

Engine split: DVE sq_a/red_a/sq_i/red_b/scale_a/scale_b/red_i then the
esum chain (diag reduces dep-forced into loop slack); GP sq_b + diag
mults; ACT only ln/exp stat scalars + the 8 big EXPs + cs copy.
"""

import numpy as np

B = 4096
D = 128
P = 128
NCORES = 8
RB = 1024                  # rows per core
CB = 2048                  # cols per core
RT = RB // P               # 8 row tiles
TEMP = 0.25
LN4 = float(np.log(4.0))

_cache = {}


def _build_bass():
    import concourse.bass as bass
    import concourse.mybir as mybir
    import concourse.tile as tile
    from concourse.bass import broadcast_tensor_aps
    from concourse.tile_rust import add_dep_helper

    f32 = mybir.dt.float32
    bf16 = mybir.dt.bfloat16
    AF = mybir.ActivationFunctionType
    ALU = mybir.AluOpType
    AX = mybir.AxisListType

    nc = bass.Bass("TRN2")
    ai_d = nc.dram_tensor("emb_i_blk", [RB, D], bf16, kind="ExternalInput")
    ca_d = nc.dram_tensor("emb_j_ca", [RB, D], bf16, kind="ExternalInput")
    cb_d = nc.dram_tensor("emb_j_cb", [RB, D], bf16, kind="ExternalInput")
    out_rs = nc.dram_tensor("rowsum", [P, RT], f32, kind="ExternalOutput")
    out_cs = nc.dram_tensor("colsum", [P, 512], f32, kind="ExternalOutput")
    out_dg = nc.dram_tensor("diag", [P, 5 * RT], f32, kind="ExternalOutput")

    ai_t = ai_d.rearrange("(p t) d -> p t d", p=P)   # row g = p*8 + t
    ca_t = ca_d.rearrange("(p t) d -> p t d", p=P)   # col g = p*8 + t
    cb_t = cb_d.rearrange("(p t) d -> p t d", p=P)   # col g = 1024 + p*8 + t

    with tile.TileContext(nc) as tc:
        with (
            tc.tile_pool(name="persist", bufs=1) as persist,
            tc.tile_pool(name="scratch", bufs=4) as scratch,
            tc.tile_pool(name="ebuf", bufs=2) as ebuf,
            tc.tile_pool(name="psmain", bufs=2, space="PSUM") as psmain,
        ):
            aib = persist.tile([P, RT, D], bf16, tag="aib")
            cja = persist.tile([P, RT, D], bf16, tag="cja")
            cjb = persist.tile([P, RT, D], bf16, tag="cjb")
            aibT = persist.tile([P, RT, D], bf16, tag="aibT")
            sqi = persist.tile([P, RT, D], bf16, tag="sqi")
            sqa = persist.tile([P, RT, D], bf16, tag="sqa")
            sqb = persist.tile([P, RT, D], bf16, tag="sqb")
            zca = persist.tile([P, RT, D], bf16, tag="zca")
            zcb = persist.tile([P, RT, D], bf16, tag="zcb")
            zcaT = persist.tile([P, RT, D], bf16, tag="zcaT")
            zcbT = persist.tile([P, RT, D], bf16, tag="zcbT")
            n2a = persist.tile([P, RT], f32, tag="n2a")
            n2b = persist.tile([P, RT], f32, tag="n2b")
            n2i = persist.tile([P, RT], f32, tag="n2i")
            inv4a = persist.tile([P, RT, 1], bf16, tag="inv4a")
            inv4b = persist.tile([P, RT, 1], bf16, tag="inv4b")
            dg = persist.tile([P, 5 * RT], f32, tag="dg")
            rs_sb = persist.tile([P, RT], f32, tag="rs_sb")
            cs_sb = persist.tile([P, 512], f32, tag="cs_sb")
            esum = [
                persist.tile([P, CB], bf16, name="esum0", tag="esum0"),
                persist.tile([P, CB], bf16, name="esum1", tag="esum1"),
            ]
            zb = persist.tile([P, 1], f32, tag="zb")
            b_ln4 = persist.tile([P, 1], f32, tag="b_ln4")
            mask2 = persist.tile([P, 2], bf16, tag="mask2")
            dxin = persist.tile([16, D], bf16, tag="dxin")
            dxout = persist.tile([P, 16], bf16, tag="dxout")

            rva = dg[:, 0:RT]
            rvb = dg[:, RT:2 * RT]
            invi = dg[:, 2 * RT:3 * RT]
            n2c_out = dg[:, 3 * RT:5 * RT]       # [n2a | n2b]

            # ---- loads: cj_b first (longest chain), two DGE queues ----
            nc.sync.dma_start(out=cjb, in_=cb_t)
            nc.scalar.dma_start(out=cja, in_=ca_t)
            nc.sync.dma_start(out=aib, in_=ai_t)

            # ---- tiny constants ----
            nc.vector.memset(zb, 0.0)
            nc.vector.memset(b_ln4, LN4)
            nc.vector.memset(mask2, 1.0)
            nc.vector.memset(dxin, 0.0)

            dummy_inst = nc.sync.dma_start_transpose(out=dxout, in_=dxin)

            # ---- GP: square of half b (off the DVE queue) ----
            nc.gpsimd.tensor_mul(sqb, cjb, cjb)

            # ---- DVE stats/scales (order = queue order) ----
            nc.vector.tensor_mul(sqa, cja, cja)                    # 2x
            nc.vector.tensor_reduce(out=n2a, in_=sqa, axis=AX.X, op=ALU.add)
            nc.vector.tensor_mul(sqi, aib, aib)                    # 2x
            nc.vector.tensor_reduce(out=n2b, in_=sqb, axis=AX.X, op=ALU.add)

            lga = scratch.tile([P, RT], f32, tag="lga")
            nc.scalar.activation(lga, n2a, AF.Ln, bias=zb)
            nc.scalar.activation(inv4a[:, :, 0], lga, AF.Exp,
                                 scale=-0.5, bias=b_ln4)
            lgb = scratch.tile([P, RT], f32, tag="lgb")
            nc.scalar.activation(lgb, n2b, AF.Ln, bias=zb)
            nc.scalar.activation(inv4b[:, :, 0], lgb, AF.Exp,
                                 scale=-0.5, bias=b_ln4)

            a_ap, b_ap = broadcast_tensor_aps(cja, inv4a)
            nc.vector.tensor_tensor(out=zca, in0=a_ap, in1=b_ap, op=ALU.mult)
            a_ap, b_ap = broadcast_tensor_aps(cjb, inv4b)
            sc_b = nc.vector.tensor_tensor(out=zcb, in0=a_ap, in1=b_ap,
                                           op=ALU.mult)
            red_i = nc.vector.tensor_reduce(out=n2i, in_=sqi, axis=AX.X,
                                            op=ALU.add)

            lgi = scratch.tile([P, RT], f32, tag="lgi")
            nc.scalar.activation(lgi, n2i, AF.Ln, bias=zb)
            nc.scalar.activation(invi, lgi, AF.Exp, scale=-0.5, bias=zb)

            # n2c ships to host (diagonal needs |z_j| of own rows)
            cp_n = nc.vector.tensor_copy(n2c_out[:, 0:RT], n2a)
            nc.vector.tensor_copy(n2c_out[:, RT:2 * RT], n2b)

            # ---- transposes (xbar serial): aibT asap, then halves ----
            t1 = nc.sync.dma_start_transpose(out=aibT, in_=aib)
            add_dep_helper(t1.ins, dummy_inst.ins, False, "xpose after dummy")
            t2 = nc.sync.dma_start_transpose(out=zcbT, in_=zcb)
            add_dep_helper(t2.ins, dummy_inst.ins, False, "xpose after dummy")
            t3 = nc.sync.dma_start_transpose(out=zcaT, in_=zca)
            add_dep_helper(t3.ins, dummy_inst.ins, False, "xpose after dummy")

            # ---- diag elementwise on GPSIMD ----
            dda = scratch.tile([P, RT, D], bf16, tag="dda")
            ddb = scratch.tile([P, RT, D], bf16, tag="ddb")
            nc.gpsimd.tensor_mul(dda, aib, cja)
            nc.gpsimd.tensor_mul(ddb, aib, cjb)

            zcaT_f = zcaT.rearrange("p t d -> p (t d)")
            zcbT_f = zcbT.rearrange("p t d -> p (t d)")
            movs = [zcaT_f[:, 0:512], zcaT_f[:, 512:1024],
                    zcbT_f[:, 0:512], zcbT_f[:, 512:1024]]

            # ---- main loop: 8 chunks (one row tile each) ----
            eb_last = None
            for k in range(RT):
                ps = psmain.tile([P, CB], f32, tag="ps")
                for q in range(4):
                    nc.tensor.matmul(
                        ps[:, q * 512:(q + 1) * 512],
                        aibT[:, k, :],
                        movs[q],
                        start=True,
                        stop=True,
                    )
                eb = ebuf.tile([P, CB], bf16, tag="eb")
                eb_last = eb
                nc.scalar.activation(
                    eb, ps, AF.Exp,
                    scale=invi[:, k:k + 1],
                    bias=zb,
                    accum_out=rs_sb[:, k:k + 1],
                )
                if k == 0:
                    nc.vector.tensor_copy(esum[0], eb)        # 4x
                elif k < RT - 1:
                    nc.vector.tensor_tensor(
                        out=esum[k % 2], in0=eb, in1=esum[(k + 1) % 2],
                        op=ALU.add,
                    )                                          # 2x
                if k == 2:
                    # diag reduces into DVE loop slack; dep-forced after
                    # red_i so the scheduler can't hoist them earlier
                    r1 = nc.vector.tensor_reduce(out=rva, in_=dda,
                                                 axis=AX.X, op=ALU.add)
                    add_dep_helper(r1.ins, red_i.ins, False, "diag late")
                if k == 4:
                    r2 = nc.vector.tensor_reduce(out=rvb, in_=ddb,
                                                 axis=AX.X, op=ALU.add)
                    add_dep_helper(r2.ins, red_i.ins, False, "diag late")

            es_fin = esum[(RT - 2) % 2]     # chain through chunk 6

            # ---- tail: colsum partials via accumulating mask matmuls;
            # q-th block on PSUM partitions 32q, free 0:512 ----
            psR_full = psmain.tile([P, CB], f32, tag="ps")
            for q in range(4):
                nc.tensor.matmul(
                    psR_full[32 * q:32 * q + 2, 0:512],
                    mask2,
                    es_fin[:, q * 512:(q + 1) * 512],
                    start=True,
                    stop=False,
                    tile_position=(0, 32 * q),
                )
            for q in range(4):
                nc.tensor.matmul(
                    psR_full[32 * q:32 * q + 2, 0:512],
                    mask2,
                    eb_last[:, q * 512:(q + 1) * 512],
                    start=False,
                    stop=True,
                    tile_position=(0, 32 * q),
                )
            nc.scalar.activation(cs_sb, psR_full[:, 0:512], AF.Copy)

            nc.sync.dma_start(out=out_dg[:, :], in_=dg)
            nc.sync.dma_start(out=out_rs[:, :], in_=rs_sb)
            nc.sync.dma_start(out=out_cs[:, :], in_=cs_sb)

    return nc


def _split_multi_waits(bir: bytes) -> bytes:
    """The walrus build in this container accepts only ONE sync-wait per
    compute/DMA instruction. Tile emits up to three. Move all but one wait
    onto standalone EventSemaphore instructions inserted just before the
    offender on the same engine queue."""
    import json

    d = json.loads(bir)
    n_split = 0
    for fn in d["functions"]:
        for blk in fn["blocks"]:
            new_insts = []
            for ins in blk["instructions"]:
                si = ins.get("sync_info")
                waits = (si or {}).get("on_wait") or []
                if len(waits) > 1:
                    for w in waits[:-1]:
                        ev = {
                            "debug": ins.get("debug", 0),
                            "engine": ins["engine"],
                            "ins": [],
                            "outs": [],
                            "name": f"{ins['name']}_wsplit{n_split}",
                            "opcode": "EventSemaphore",
                            "sync_info": {"on_update": [], "on_wait": [w]},
                        }
                        n_split += 1
                        new_insts.append(ev)
                    si["on_wait"] = [waits[-1]]
                new_insts.append(ins)
            blk["instructions"] = new_insts
    return json.dumps(d).encode()


def kernel(emb_i: np.ndarray, emb_j: np.ndarray) -> np.ndarray:
    import ml_dtypes
    from concourse.bass_utils import run_bass_kernel_spmd

    if "nc" not in _cache:
        nc = _build_bass()
        fixed = _split_multi_waits(nc.to_json_bytes())
        nc.to_json_bytes = lambda: fixed
        _cache["nc"] = nc
    nc = _cache["nc"]

    bf = ml_dtypes.bfloat16
    emb_i = np.ascontiguousarray(emb_i, dtype=np.float32).astype(bf)
    emb_j = np.ascontiguousarray(emb_j, dtype=np.float32).astype(bf)
    in_maps = []
    for c in range(NCORES):
        rb, ch = c // 2, c % 2
        in_maps.append(
            {
                "emb_i_blk": emb_i[rb * RB:(rb + 1) * RB],
                "emb_j_ca": emb_j[ch * CB:ch * CB + RB],
                "emb_j_cb": emb_j[ch * CB + RB:(ch + 1) * CB],
            }
        )

    import os

    trace = bool(os.environ.get("KERNEL_TRACE"))
    res = run_bass_kernel_spmd(
        nc, in_maps, core_ids=list(range(NCORES)), trace=trace
    )
    _cache["last_res"] = res

    # ---- host combine ----
    rs_total = np.zeros(B, dtype=np.float64)
    cs_total = np.zeros(B, dtype=np.float64)
    dtot = np.float64(0.0)
    for c, r in enumerate(res.results):
        rb, ch = c // 2, c % 2
        # rowsum [128, 8]: (p, k) -> local row p*8+k
        rs_total[rb * RB:(rb + 1) * RB] += (
            r["rowsum"].astype(np.float64).reshape(RB)
        )
        # colsum [128, 512]: q-th 512-block on partitions 32q..32q+1;
        # block free j = t_l*128 + p -> local col
        #   1024*(q//2) + p*8 + 4*(q%2) + t_l
        co = r["colsum"].astype(np.float64)
        for q in range(4):
            blk = 0.5 * (co[32 * q] + co[32 * q + 1])        # [512]
            half, sub = q // 2, q % 2
            dst = cs_total[ch * CB + half * RB:ch * CB + half * RB + RB]
            dst.reshape(P, 2, 4)[:, sub, :] += blk.reshape(4, P).T
        if rb // 2 == ch:
            # this core's emb_i row block lies inside its cj col block
            d = r["diag"].astype(np.float64)
            iv = d[:, 2 * RT:3 * RT].reshape(RB)
            half = rb % 2
            rv = d[:, half * RT:(half + 1) * RT].reshape(RB)
            n2o = d[:, (3 + half) * RT:(4 + half) * RT].reshape(RB)
            # pos/T = 4 * rvec * invi / sqrt(n2o); contributes -2*pos/T
            dtot += np.sum(-8.0 * rv * iv / np.sqrt(n2o))
    total = dtot + np.log(rs_total).sum() + np.log(cs_total).sum()
    loss = total / (2 * B)
    return np.array(loss, dtype=np.float32)


# revision 22
# speedup vs baseline: 1.3808x; 1.0489x over previous
"""Trainium2 Bass kernel for NT-Xent style contrastive loss (v5).

Math (B=4096, D=128, T=0.25), z = row-normalized emb:
  S = z_i @ z_j^T   [B, B]
  loss = (1/2B) * sum_r [ -2*S[r,r]/T + ln(sum_c exp(S[r,c]/T))
                                      + ln(sum_c exp(S[c,r]/T)) ]

Sharding: core (rb, ch), rb = core//2, ch = core%2: rows rb*1024 of emb_i,
cols ch*2048 of emb_j.  Inputs are cast to bf16 ON THE HOST (input
quantization, ~0.4% per element, far inside the 2e-2 tolerance): halves
the DMA wire time, feeds the matmuls directly, and makes every stats op
2x-mode eligible.  All tensors use the "(p t) d" 8-row tiling (partition
g//8, tile g%8, 2KB contiguous descriptors); outputs are unpermuted on
the host.  emb_j's block is loaded as two 1024-row halves with the same
tiling as emb_i's block, so the diagonal dot products are computed
against cj directly (each row block equals one cj half on cores
0/2/5/7) and the column norms n2c double as the diagonal |z_j| norms.

Orientation: ps chunk k = [128 r (row-tile k), 2048 c free], 8 chunks.
  stationary = aibT tile k (RAW bf16 emb_i, transposed straight off the
               load; row stats are off the matmul critical path)
  moving     = zcjT halves (cols pre-scaled by 4/|c|, absorbing 1/T)
The EXP applies the row norm via its per-partition scale AP
(scale = invi[:,k]) and its accumulator emits the ROW SUMS directly
(accum_out -> rs[:, k]).  Column-sum partials: bf16 TT esum chain
(DVE 2x, chunks 1-6) + accumulating mask matmuls over {esum, eb_7},
stacked on PSUM partitions 32q so one free-512 ACT copy extracts them.

Engine split: DVE sq_a/red_a/sq_i/red_b/scale_a/scale_b/red_i then the
esum chain (diag reduces dep-forced into loop slack); GP sq_b + diag
mults; ACT only ln/exp stat scalars + the 8 big EXPs + cs copy.
"""

import numpy as np

B = 4096
D = 128
P = 128
NCORES = 8
RB = 1024                  # rows per core
CB = 2048                  # cols per core
RT = RB // P               # 8 row tiles
TEMP = 0.25
LN4 = float(np.log(4.0))

_cache = {}


def _build_bass():
    import concourse.bass as bass
    import concourse.mybir as mybir
    import concourse.tile as tile
    from concourse.bass import broadcast_tensor_aps
    from concourse.tile_rust import add_dep_helper

    f32 = mybir.dt.float32
    bf16 = mybir.dt.bfloat16
    AF = mybir.ActivationFunctionType
    ALU = mybir.AluOpType
    AX = mybir.AxisListType

    nc = bass.Bass("TRN2")
    ai_d = nc.dram_tensor("emb_i_blk", [RB, D], bf16, kind="ExternalInput")
    ca_d = nc.dram_tensor("emb_j_ca", [RB, D], bf16, kind="ExternalInput")
    cb_d = nc.dram_tensor("emb_j_cb", [RB, D], bf16, kind="ExternalInput")
    out_rs = nc.dram_tensor("rowsum", [P, RT], f32, kind="ExternalOutput")
    out_cs = nc.dram_tensor("colsum", [P, 512], bf16, kind="ExternalOutput")
    out_dg = nc.dram_tensor("diag", [P, 5 * RT], f32, kind="ExternalOutput")

    ai_t = ai_d.rearrange("(p t) d -> p t d", p=P)   # row g = p*8 + t
    ca_t = ca_d.rearrange("(p t) d -> p t d", p=P)   # col g = p*8 + t
    cb_t = cb_d.rearrange("(p t) d -> p t d", p=P)   # col g = 1024 + p*8 + t

    with tile.TileContext(nc) as tc:
        with (
            tc.tile_pool(name="persist", bufs=1) as persist,
            tc.tile_pool(name="scratch", bufs=4) as scratch,
            tc.tile_pool(name="ebuf", bufs=2) as ebuf,
            tc.tile_pool(name="psmain", bufs=2, space="PSUM") as psmain,
        ):
            aib = persist.tile([P, RT, D], bf16, tag="aib")
            cja = persist.tile([P, RT, D], bf16, tag="cja")
            cjb = persist.tile([P, RT, D], bf16, tag="cjb")
            aibT = persist.tile([P, RT, D], bf16, tag="aibT")
            sqi = persist.tile([P, RT, D], bf16, tag="sqi")
            sqa = persist.tile([P, RT, D], bf16, tag="sqa")
            sqb = persist.tile([P, RT, D], bf16, tag="sqb")
            zca = persist.tile([P, RT, D], bf16, tag="zca")
            zcb = persist.tile([P, RT, D], bf16, tag="zcb")
            zcaT = persist.tile([P, RT, D], bf16, tag="zcaT")
            zcbT = persist.tile([P, RT, D], bf16, tag="zcbT")
            n2a = persist.tile([P, RT], f32, tag="n2a")
            n2b = persist.tile([P, RT], f32, tag="n2b")
            n2i = persist.tile([P, RT], f32, tag="n2i")
            inv4a = persist.tile([P, RT, 1], bf16, tag="inv4a")
            inv4b = persist.tile([P, RT, 1], bf16, tag="inv4b")
            dg = persist.tile([P, 5 * RT], f32, tag="dg")
            rs_sb = persist.tile([P, RT], f32, tag="rs_sb")
            cs_sb = persist.tile([P, 512], bf16, tag="cs_sb")
            esum = [
                persist.tile([P, CB], bf16, name="esum0", tag="esum0"),
                persist.tile([P, CB], bf16, name="esum1", tag="esum1"),
            ]
            zb = persist.tile([P, 1], f32, tag="zb")
            b_ln4 = persist.tile([P, 1], f32, tag="b_ln4")
            mask2 = persist.tile([P, 2], bf16, tag="mask2")
            dxin = persist.tile([16, D], bf16, tag="dxin")
            dxout = persist.tile([P, 16], bf16, tag="dxout")

            rva = dg[:, 0:RT]
            rvb = dg[:, RT:2 * RT]
            invi = dg[:, 2 * RT:3 * RT]
            n2c_out = dg[:, 3 * RT:5 * RT]       # [n2a | n2b]

            # ---- loads: cj_b first (longest chain), two DGE queues ----
            nc.sync.dma_start(out=cjb, in_=cb_t)
            nc.scalar.dma_start(out=cja, in_=ca_t)
            nc.sync.dma_start(out=aib, in_=ai_t)

            # ---- tiny constants ----
            nc.vector.memset(zb, 0.0)
            nc.vector.memset(b_ln4, LN4)
            nc.vector.memset(mask2, 1.0)
            nc.vector.memset(dxin, 0.0)

            dummy_inst = nc.sync.dma_start_transpose(out=dxout, in_=dxin)

            # ---- squares on ACT (a DVE TT squaring one operand runs at
            # half rate; ACT is idle in the preamble) ----
            nc.scalar.activation(sqa, cja, AF.Square)
            nc.scalar.activation(sqb, cjb, AF.Square)

            # ---- DVE reds + scales (order = queue order) ----
            nc.vector.tensor_reduce(out=n2a, in_=sqa, axis=AX.X, op=ALU.add)
            nc.vector.tensor_reduce(out=n2b, in_=sqb, axis=AX.X, op=ALU.add)

            lga = scratch.tile([P, RT], f32, tag="lga")
            nc.scalar.activation(lga, n2a, AF.Ln, bias=zb)
            nc.scalar.activation(inv4a[:, :, 0], lga, AF.Exp,
                                 scale=-0.5, bias=b_ln4)
            lgb = scratch.tile([P, RT], f32, tag="lgb")
            nc.scalar.activation(lgb, n2b, AF.Ln, bias=zb)
            nc.scalar.activation(inv4b[:, :, 0], lgb, AF.Exp,
                                 scale=-0.5, bias=b_ln4)

            a_ap, b_ap = broadcast_tensor_aps(cja, inv4a)
            nc.vector.tensor_tensor(out=zca, in0=a_ap, in1=b_ap, op=ALU.mult)
            a_ap, b_ap = broadcast_tensor_aps(cjb, inv4b)
            nc.vector.tensor_tensor(out=zcb, in0=a_ap, in1=b_ap, op=ALU.mult)

            nc.scalar.activation(sqi, aib, AF.Square)
            red_i = nc.vector.tensor_reduce(out=n2i, in_=sqi, axis=AX.X,
                                            op=ALU.add)

            lgi = scratch.tile([P, RT], f32, tag="lgi")
            nc.scalar.activation(lgi, n2i, AF.Ln, bias=zb)
            nc.scalar.activation(invi, lgi, AF.Exp, scale=-0.5, bias=zb)

            # n2c ships to host (diagonal needs |z_j| of own rows)
            cp_n = nc.vector.tensor_copy(n2c_out[:, 0:RT], n2a)
            nc.vector.tensor_copy(n2c_out[:, RT:2 * RT], n2b)

            # ---- transposes (xbar serial): aibT asap, then halves ----
            t1 = nc.sync.dma_start_transpose(out=aibT, in_=aib)
            add_dep_helper(t1.ins, dummy_inst.ins, False, "xpose after dummy")
            t2 = nc.sync.dma_start_transpose(out=zcbT, in_=zcb)
            add_dep_helper(t2.ins, dummy_inst.ins, False, "xpose after dummy")
            t3 = nc.sync.dma_start_transpose(out=zcaT, in_=zca)
            add_dep_helper(t3.ins, dummy_inst.ins, False, "xpose after dummy")

            # ---- diag elementwise on GPSIMD ----
            dda = scratch.tile([P, RT, D], bf16, tag="dda")
            ddb = scratch.tile([P, RT, D], bf16, tag="ddb")
            nc.gpsimd.tensor_mul(dda, aib, cja)
            nc.gpsimd.tensor_mul(ddb, aib, cjb)

            zcaT_f = zcaT.rearrange("p t d -> p (t d)")
            zcbT_f = zcbT.rearrange("p t d -> p (t d)")
            movs = [zcaT_f[:, 0:512], zcaT_f[:, 512:1024],
                    zcbT_f[:, 0:512], zcbT_f[:, 512:1024]]

            # ---- main loop: 8 chunks (one row tile each) ----
            eb_last = None
            for k in range(RT):
                ps = psmain.tile([P, CB], f32, tag="ps")
                for q in range(4):
                    nc.tensor.matmul(
                        ps[:, q * 512:(q + 1) * 512],
                        aibT[:, k, :],
                        movs[q],
                        start=True,
                        stop=True,
                    )
                eb = ebuf.tile([P, CB], bf16, tag="eb")
                eb_last = eb
                nc.scalar.activation(
                    eb, ps, AF.Exp,
                    scale=invi[:, k:k + 1],
                    bias=zb,
                    accum_out=rs_sb[:, k:k + 1],
                )
                if k == 0:
                    nc.vector.tensor_copy(esum[0], eb)        # 4x
                elif k < RT - 1:
                    nc.vector.tensor_tensor(
                        out=esum[k % 2], in0=eb, in1=esum[(k + 1) % 2],
                        op=ALU.add,
                    )                                          # 2x
                if k == 2:
                    # diag reduces into DVE loop slack; dep-forced after
                    # red_i so the scheduler can't hoist them earlier
                    r1 = nc.vector.tensor_reduce(out=rva, in_=dda,
                                                 axis=AX.X, op=ALU.add)
                    add_dep_helper(r1.ins, red_i.ins, False, "diag late")
                if k == 4:
                    r2 = nc.vector.tensor_reduce(out=rvb, in_=ddb,
                                                 axis=AX.X, op=ALU.add)
                    add_dep_helper(r2.ins, red_i.ins, False, "diag late")

            es_fin = esum[(RT - 2) % 2]     # chain through chunk 6

            # ---- tail: colsum partials via accumulating mask matmuls;
            # q-th block on PSUM partitions 32q, free 0:512 ----
            psR_full = psmain.tile([P, CB], f32, tag="ps")
            for q in range(4):
                nc.tensor.matmul(
                    psR_full[32 * q:32 * q + 2, 0:512],
                    mask2,
                    es_fin[:, q * 512:(q + 1) * 512],
                    start=True,
                    stop=False,
                    tile_position=(0, 32 * q),
                )
            for q in range(4):
                nc.tensor.matmul(
                    psR_full[32 * q:32 * q + 2, 0:512],
                    mask2,
                    eb_last[:, q * 512:(q + 1) * 512],
                    start=False,
                    stop=True,
                    tile_position=(0, 32 * q),
                )
            nc.scalar.activation(cs_sb, psR_full[:, 0:512], AF.Copy)  # ->bf16

            nc.sync.dma_start(out=out_dg[:, :], in_=dg)
            nc.sync.dma_start(out=out_rs[:, :], in_=rs_sb)
            nc.sync.dma_start(out=out_cs[:, :], in_=cs_sb)

    return nc


def _split_multi_waits(bir: bytes) -> bytes:
    """The walrus build in this container accepts only ONE sync-wait per
    compute/DMA instruction. Tile emits up to three. Move all but one wait
    onto standalone EventSemaphore instructions inserted just before the
    offender on the same engine queue."""
    import json

    d = json.loads(bir)
    n_split = 0
    for fn in d["functions"]:
        for blk in fn["blocks"]:
            new_insts = []
            for ins in blk["instructions"]:
                si = ins.get("sync_info")
                waits = (si or {}).get("on_wait") or []
                if len(waits) > 1:
                    for w in waits[:-1]:
                        ev = {
                            "debug": ins.get("debug", 0),
                            "engine": ins["engine"],
                            "ins": [],
                            "outs": [],
                            "name": f"{ins['name']}_wsplit{n_split}",
                            "opcode": "EventSemaphore",
                            "sync_info": {"on_update": [], "on_wait": [w]},
                        }
                        n_split += 1
                        new_insts.append(ev)
                    si["on_wait"] = [waits[-1]]
                new_insts.append(ins)
            blk["instructions"] = new_insts
    return json.dumps(d).encode()


def kernel(emb_i: np.ndarray, emb_j: np.ndarray) -> np.ndarray:
    import ml_dtypes
    from concourse.bass_utils import run_bass_kernel_spmd

    if "nc" not in _cache:
        nc = _build_bass()
        fixed = _split_multi_waits(nc.to_json_bytes())
        nc.to_json_bytes = lambda: fixed
        _cache["nc"] = nc
    nc = _cache["nc"]

    bf = ml_dtypes.bfloat16
    emb_i = np.ascontiguousarray(emb_i, dtype=np.float32).astype(bf)
    emb_j = np.ascontiguousarray(emb_j, dtype=np.float32).astype(bf)
    in_maps = []
    for c in range(NCORES):
        rb, ch = c // 2, c % 2
        in_maps.append(
            {
                "emb_i_blk": emb_i[rb * RB:(rb + 1) * RB],
                "emb_j_ca": emb_j[ch * CB:ch * CB + RB],
                "emb_j_cb": emb_j[ch * CB + RB:(ch + 1) * CB],
            }
        )

    import os

    trace = bool(os.environ.get("KERNEL_TRACE"))
    res = run_bass_kernel_spmd(
        nc, in_maps, core_ids=list(range(NCORES)), trace=trace
    )
    _cache["last_res"] = res

    # ---- host combine ----
    rs_total = np.zeros(B, dtype=np.float64)
    cs_total = np.zeros(B, dtype=np.float64)
    dtot = np.float64(0.0)
    for c, r in enumerate(res.results):
        rb, ch = c // 2, c % 2
        # rowsum [128, 8]: (p, k) -> local row p*8+k
        rs_total[rb * RB:(rb + 1) * RB] += (
            r["rowsum"].astype(np.float64).reshape(RB)
        )
        # colsum [128, 512]: q-th 512-block on partitions 32q..32q+1;
        # block free j = t_l*128 + p -> local col
        #   1024*(q//2) + p*8 + 4*(q%2) + t_l
        co = r["colsum"].astype(np.float64)
        for q in range(4):
            blk = 0.5 * (co[32 * q] + co[32 * q + 1])        # [512]
            half, sub = q // 2, q % 2
            dst = cs_total[ch * CB + half * RB:ch * CB + half * RB + RB]
            dst.reshape(P, 2, 4)[:, sub, :] += blk.reshape(4, P).T
        if rb // 2 == ch:
            # this core's emb_i row block lies inside its cj col block
            d = r["diag"].astype(np.float64)
            iv = d[:, 2 * RT:3 * RT].reshape(RB)
            half = rb % 2
            rv = d[:, half * RT:(half + 1) * RT].reshape(RB)
            n2o = d[:, (3 + half) * RT:(4 + half) * RT].reshape(RB)
            # pos/T = 4 * rvec * invi / sqrt(n2o); contributes -2*pos/T
            dtot += np.sum(-8.0 * rv * iv / np.sqrt(n2o))
    total = dtot + np.log(rs_total).sum() + np.log(cs_total).sum()
    loss = total / (2 * B)
    return np.array(loss, dtype=np.float32)
